# revision 1
# baseline (speedup 1.0000x reference)
"""Trainium2 Bass kernel for a 2-layer GCN (GCNConv -> ReLU -> Linear).

Math (matching the PyG-style reference):
    deg  = in_degree(dst) + 1 (self loops), dinv = deg^-1/2
    h    = X @ W1                                  [N, 64]
    agg[d] = dinv[d] * sum_{e:(s->d)} dinv[s]*h[s] (+ self loop)   [N, 64]
    out  = relu(agg + b1) @ Wfc.T + bfc            [N, 40]

Distribution over 8 NeuronCores (graph/data parallel):
  - Nodes are re-labeled into 392 "tiles" of 128 slots (balanced by degree),
    49 tiles per core.  Each core computes h' = dinv*h for its 6272 slots
    (X @ W1 on the tensor engine), writes them as bf16 rows padded to 256 B,
    and an AllGather replicates the full 50176-row table into every core's
    HBM.
  - Each core aggregates the edges whose destination it owns: a bulk
    SWDGE dma_gather fetches h'[src] rows (256 B each) into SBUF in
    edge-major layout; destinations are scatter-added via one-hot matmuls
    (lhsT = gathered messages [128 edges, 64], rhs = C [128 edges, 128 dst]
    with C[j, d] = dinv_dst[j] * (dst_loc[j] == d)) accumulated in PSUM.
    C is built on the vector engine with a single fused tensor_scalar
    (is_equal then mult) per 128-edge chunk.
  - relu(agg + b1) is fused into the PSUM->SBUF eviction on the scalar
    engine (per-partition bias = b1 since agg is kept feature-major), the
    second layer is one small matmul per tile, and the bfc bias rides the
    final eviction.  The host un-permutes the [40, slots] outputs.

dma_gather indices are int16, so the row table is split at row 32768 into a
"lo" and a "hi" region, and each tile's edges are packed into K_LO lo-chunks
followed by K_HI hi-chunks (pad lanes: idx=0, dst_loc=-1, dinv_dst=0).
"""

import numpy as np

# ----------------------------------------------------------------------------
# Problem configuration (hardcoded; kernel.py must be self-contained).
# ----------------------------------------------------------------------------
N_NODES = 50000
N_EDGES = 800000
IN_DIM = 512
HID = 64
OUT_DIM = 40
N_CORES = 8

# Gather-call shape: ≤GATHER_MAX_CHUNKS*128 indices per dma_gather call.
# single_packet=True is limited to 64 descriptors (1024 idxs) per call and
# serializes each call onto one SDMA engine.
GATHER_MAX_CHUNKS = 18
GATHER_SINGLE_PACKET = False


class Cfg:
    def __init__(self, n_nodes, in_dim, hid, out_dim, n_cores, tiles_per_core,
                 group, lo_boundary, k_lo, k_hi):
        self.n_nodes = n_nodes
        self.in_dim = in_dim
        self.hid = hid
        self.out_dim = out_dim
        self.n_cores = n_cores
        self.nt = tiles_per_core              # tiles per core
        self.group = group                    # tiles per gather group
        assert self.nt % self.group == 0
        self.n_groups = self.nt // self.group
        self.slots_per_core = self.nt * 128
        self.n_tiles = n_cores * self.nt
        self.n_slots = self.n_tiles * 128
        self.lo_b = lo_boundary               # table split row (<= 32768)
        self.k_lo = k_lo                      # lo chunks per tile
        self.k_hi = k_hi                      # hi chunks per tile
        self.k = k_lo + k_hi
        self.kd = in_dim // 128               # contraction tiles for X @ W1
        assert in_dim % 128 == 0
        assert self.n_slots - self.lo_b <= 32768 and self.lo_b <= 32768


# ----------------------------------------------------------------------------
# Host-side graph preprocessing (index/layout work only; all feature math
# runs on the device).
# ----------------------------------------------------------------------------

def _plan(edges, cfg_base):
    """Relabel nodes into balanced tiles and pack edges into chunk slots.

    Returns (cfg, plan dict).  cfg_base is (n_nodes, in_dim, hid, out_dim,
    n_cores, nt, group, lo_boundary); k_lo/k_hi are derived from the data.
    """
    n_nodes, in_dim, hid, out_dim, n_cores, nt, group, lo_b = cfg_base
    n_tiles = n_cores * nt
    n_slots = n_tiles * 128

    src = np.asarray(edges[0], dtype=np.int64)
    dst = np.asarray(edges[1], dtype=np.int64)
    deg = np.bincount(dst, minlength=n_nodes).astype(np.int64) + 1
    dinv = (1.0 / np.sqrt(deg.astype(np.float64))).astype(np.float32)

    # Snake binpack nodes (by degree, desc) into n_tiles bins of <=128 slots.
    order = np.argsort(-deg, kind="stable")
    rounds = np.arange(n_nodes) // n_tiles
    pos = np.arange(n_nodes) % n_tiles
    tile_of = np.where(rounds % 2 == 0, pos, n_tiles - 1 - pos)
    assert rounds.max() < 128, "more than 128 slots per tile"
    node_to_slot = np.empty(n_nodes, dtype=np.int64)
    node_to_slot[order] = tile_of * 128 + rounds

    slot_dinv = np.zeros(n_slots, dtype=np.float32)
    slot_dinv[node_to_slot] = dinv

    # Full edge list including self loops, in slot space.
    s_slot = np.concatenate([node_to_slot[src], node_to_slot])
    d_slot = np.concatenate([node_to_slot[dst], node_to_slot])
    d_tile = d_slot >> 7
    is_hi = (s_slot >= lo_b).astype(np.int64)

    # Group edges by (dst tile, lo/hi class).
    key = d_tile * 2 + is_hi
    sort_idx = np.argsort(key, kind="stable")
    key_s = key[sort_idx]
    s_slot_s = s_slot[sort_idx]
    d_slot_s = d_slot[sort_idx]
    counts = np.bincount(key_s, minlength=n_tiles * 2)
    starts = np.concatenate([[0], np.cumsum(counts)[:-1]])
    rank_in_group = np.arange(len(key_s)) - starts[key_s]

    n_lo = counts[0::2]
    n_hi = counts[1::2]
    k_lo = int(np.max((n_lo + 127) // 128)) if n_lo.max() > 0 else 0
    k_hi = int(np.max((n_hi + 127) // 128)) if n_hi.max() > 0 else 0
    k_lo = max(k_lo, 1)
    k_hi = max(k_hi, 1)

    cfg = Cfg(n_nodes, in_dim, hid, out_dim, n_cores, nt, group, lo_b,
              k_lo, k_hi)

    # Per-core chunk-slot numbering (group-major, lo chunks then hi chunks
    # inside each group):
    #   lo: fc = g*(G*K) + i*K_LO + j
    #   hi: fc = g*(G*K) + G*K_LO + i*K_HI + j
    n_chunks_core = nt * cfg.k
    g_of_tile = (d_tile % nt) // group        # group within core
    i_of_tile = (d_tile % nt) % group         # tile within group
    core_of = d_tile // nt
    j_chunk = rank_in_group >> 7
    lane = rank_in_group & 127
    base = g_of_tile[sort_idx] * (group * cfg.k)
    fc = np.where(
        key_s % 2 == 0,
        base + i_of_tile[sort_idx] * k_lo + j_chunk,
        base + group * k_lo + i_of_tile[sort_idx] * k_hi + j_chunk,
    )
    assert (j_chunk < np.where(key_s % 2 == 0, k_lo, k_hi)).all()

    idx16 = np.zeros((n_cores, n_chunks_core, 128), dtype=np.int16)
    dstloc = np.full((n_cores, n_chunks_core, 128), -1.0, dtype=np.float32)
    dinvdst = np.zeros((n_cores, n_chunks_core, 128), dtype=np.float32)

    cidx = core_of[sort_idx]
    idx16[cidx, fc, lane] = np.where(
        key_s % 2 == 0, s_slot_s, s_slot_s - lo_b).astype(np.int16)
    dstloc[cidx, fc, lane] = (d_slot_s & 127).astype(np.float32)
    dinvdst[cidx, fc, lane] = slot_dinv[d_slot_s]

    # Wrap gather indices: per (group, class) region, list position s ->
    # partition s%16, column s//16; replicated across the 8 q7 cores
    # (128 partitions).
    n_idx_cols = n_chunks_core * 128 // 16
    idx_wrapped = np.zeros((n_cores, 128, n_idx_cols), dtype=np.int16)
    for g in range(cfg.n_groups):
        for cls in range(2):
            fc0 = g * group * cfg.k + (0 if cls == 0 else group * k_lo)
            nch = group * (k_lo if cls == 0 else k_hi)
            flat = idx16[:, fc0:fc0 + nch, :].reshape(n_cores, nch * 128)
            wrapped = flat.reshape(n_cores, nch * 8, 16).transpose(0, 2, 1)
            c0 = fc0 * 8
            idx_wrapped[:, :16, c0:c0 + nch * 8] = wrapped
    idx_wrapped[:, 16:, :] = np.tile(idx_wrapped[:, :16, :], (1, 7, 1))

    plan = dict(
        node_to_slot=node_to_slot,
        slot_dinv=slot_dinv,
        idx_wrapped=idx_wrapped,
        dstloc=dstloc.transpose(0, 2, 1).copy(),    # [cores, 128, n_chunks]
        dinvdst=dinvdst.transpose(0, 2, 1).copy(),
    )
    return cfg, plan


def _make_inputs(X, W1, b1, Wfc, bfc, cfg, plan):
    """Build the 8 per-core input dicts for run_bass_kernel_spmd."""
    import ml_dtypes
    bf16 = ml_dtypes.bfloat16
    node_to_slot = plan["node_to_slot"]
    s = cfg.slots_per_core

    Xp = np.zeros((cfg.n_slots, cfg.in_dim), dtype=np.float32)
    Xp[node_to_slot] = np.asarray(X, dtype=np.float32)

    W1r = (np.asarray(W1, dtype=np.float32)
           .reshape(cfg.kd, 128, cfg.hid).transpose(1, 0, 2)
           .reshape(128, cfg.kd * cfg.hid).astype(bf16))
    wfcT = np.ascontiguousarray(np.asarray(Wfc, dtype=np.float32).T).astype(bf16)
    b1c = np.asarray(b1, dtype=np.float32).reshape(cfg.hid, 1)
    bfcc = np.asarray(bfc, dtype=np.float32).reshape(cfg.out_dim, 1)
    iota = np.tile(np.arange(128, dtype=np.float32), (128, 1)).astype(bf16)

    in_maps = []
    for c in range(cfg.n_cores):
        xt = np.ascontiguousarray(Xp[c * s:(c + 1) * s].T).astype(bf16)
        dinv_sb = np.ascontiguousarray(
            plan["slot_dinv"][c * s:(c + 1) * s].reshape(cfg.nt, 128).T)
        in_maps.append({
            "xt": xt,
            "w1": W1r,
            "wfcT": wfcT,
            "b1": b1c,
            "bfc": bfcc,
            "iota": iota,
            "dinv_sb": dinv_sb,
            "idx": plan["idx_wrapped"][c],
            "dstloc": plan["dstloc"][c],
            "dinvdst": plan["dinvdst"][c],
        })
    return in_maps


# ----------------------------------------------------------------------------
# Device kernel.
# ----------------------------------------------------------------------------

def _build_module(cfg):
    import concourse.bass as bass
    import concourse.bacc as bacc
    import concourse.mybir as mybir
    import concourse.tile as tile
    from contextlib import ExitStack

    f32 = mybir.dt.float32
    bf16 = mybir.dt.bfloat16
    i16 = mybir.dt.int16
    S = cfg.slots_per_core
    G = cfg.group
    NCHG = G * cfg.k                      # chunks per group
    GKLO = G * cfg.k_lo                   # lo chunks per group
    n_chunks = cfg.nt * cfg.k
    n_idx_cols = n_chunks * 128 // 16

    nc = bacc.Bacc("TRN2", target_bir_lowering=False, debug=False,
                   num_devices=cfg.n_cores)

    xt_d = nc.dram_tensor("xt", [cfg.in_dim, S], bf16, kind="ExternalInput")
    w1_d = nc.dram_tensor("w1", [128, cfg.kd * cfg.hid], bf16,
                          kind="ExternalInput")
    wfcT_d = nc.dram_tensor("wfcT", [cfg.hid, cfg.out_dim], bf16,
                            kind="ExternalInput")
    b1_d = nc.dram_tensor("b1", [cfg.hid, 1], f32, kind="ExternalInput")
    bfc_d = nc.dram_tensor("bfc", [cfg.out_dim, 1], f32, kind="ExternalInput")
    iota_d = nc.dram_tensor("iota", [128, 128], bf16, kind="ExternalInput")
    dinv_d = nc.dram_tensor("dinv_sb", [128, cfg.nt], f32,
                            kind="ExternalInput")
    idx_d = nc.dram_tensor("idx", [128, n_idx_cols], i16, kind="ExternalInput")
    dstloc_d = nc.dram_tensor("dstloc", [128, n_chunks], f32,
                              kind="ExternalInput")
    dinvdst_d = nc.dram_tensor("dinvdst", [128, n_chunks], f32,
                               kind="ExternalInput")
    out_d = nc.dram_tensor("out", [cfg.out_dim, S], f32, kind="ExternalOutput")

    with tile.TileContext(nc) as tc, ExitStack() as ctx:
        dram = ctx.enter_context(tc.tile_pool(name="dram", bufs=1,
                                              space="DRAM"))
        consts = ctx.enter_context(tc.tile_pool(name="consts", bufs=1))
        ag_in = dram.tile([S, 128], bf16)
        ag_out = dram.tile([cfg.n_slots, 128], bf16)

        iota_sb = consts.tile([128, 128], bf16)
        w1_sb = consts.tile([128, cfg.kd * cfg.hid], bf16)
        wfcT_sb = consts.tile([cfg.hid, cfg.out_dim], bf16)
        b1_sb = consts.tile([cfg.hid, 1], f32)
        bfc_sb = consts.tile([cfg.out_dim, 1], f32)
        dinv_sb = consts.tile([128, cfg.nt], f32)
        idx_sb = consts.tile([128, n_idx_cols], i16)
        dstloc_sb = consts.tile([128, n_chunks], f32)
        dinvdst_sb = consts.tile([128, n_chunks], f32)

        nc.sync.dma_start(iota_sb[:], iota_d[:])
        nc.sync.dma_start(w1_sb[:], w1_d[:])
        nc.sync.dma_start(wfcT_sb[:], wfcT_d[:])
        nc.sync.dma_start(b1_sb[:], b1_d[:])
        nc.sync.dma_start(bfc_sb[:], bfc_d[:])
        nc.sync.dma_start(dinv_sb[:], dinv_d[:])
        nc.sync.dma_start(idx_sb[:], idx_d[:])
        nc.sync.dma_start(dstloc_sb[:], dstloc_d[:])
        nc.sync.dma_start(dinvdst_sb[:], dinvdst_d[:])

        # ---- Phase 1: h' = dinv * (X @ W1), bf16 rows padded to 256 B ----
        with tc.tile_pool(name="p1", bufs=1) as p1, \
                tc.tile_pool(name="p1ps", bufs=2, space="PSUM") as p1ps:
            xt_sb = p1.tile([128, cfg.kd, S], bf16)
            stage = p1.tile([128, cfg.nt, 128], bf16)
            nc.sync.dma_start(
                xt_sb[:],
                xt_d[:].rearrange("(k p) s -> p k s", p=128))
            nc.vector.memset(stage[:], 0.0)
            for t in range(cfg.nt):
                ph = p1ps.tile([128, cfg.hid], f32)
                for k in range(cfg.kd):
                    nc.tensor.matmul(
                        ph[:],
                        xt_sb[:, k, t * 128:(t + 1) * 128],
                        w1_sb[:, k * cfg.hid:(k + 1) * cfg.hid],
                        start=(k == 0), stop=(k == cfg.kd - 1))
                nc.vector.tensor_scalar_mul(
                    stage[:, t, 0:cfg.hid], ph[:],
                    dinv_sb[:, t:t + 1])
            nc.sync.dma_start(
                ag_in[:].rearrange("(t p) e -> p t e", p=128), stage[:])

        # ---- AllGather the h' table across all cores ----
        nc.gpsimd.collective_compute(
            "AllGather",
            mybir.AluOpType.bypass,
            ins=[ag_in.opt()],
            outs=[ag_out.opt()],
            replica_groups=[list(range(cfg.n_cores))],
        )

        # ---- Phase 2: gather + one-hot scatter matmuls + layer 2 ----
        msgs_p = ctx.enter_context(tc.tile_pool(name="msgs", bufs=2))
        c_p = ctx.enter_context(tc.tile_pool(name="cmat", bufs=8))
        relu_p = ctx.enter_context(tc.tile_pool(name="relu", bufs=3))
        ost_p = ctx.enter_context(tc.tile_pool(name="ost", bufs=2))
        agg_ps = ctx.enter_context(
            tc.tile_pool(name="aggps", bufs=4, space="PSUM"))
        o2_ps = ctx.enter_context(
            tc.tile_pool(name="o2ps", bufs=2, space="PSUM"))

        GMAX = GATHER_MAX_CHUNKS   # max chunks per gather call

        for g in range(cfg.n_groups):
            msgs = msgs_p.tile([128, NCHG, 128], bf16)
            col0 = g * NCHG * 8
            for r0, r1, tbl in ((0, GKLO, ag_out[0:cfg.lo_b, :]),
                                (GKLO, NCHG,
                                 ag_out[cfg.lo_b:cfg.n_slots, :])):
                cs0 = r0
                while cs0 < r1:
                    nch = min(GMAX, r1 - cs0)
                    nc.gpsimd.dma_gather(
                        msgs[:, cs0:cs0 + nch, :], tbl,
                        idx_sb[:, col0 + cs0 * 8: col0 + (cs0 + nch) * 8],
                        nch * 128, nch * 128, 128,
                        single_packet=GATHER_SINGLE_PACKET)
                    cs0 += nch

            for i in range(G):
                t = g * G + i
                agg = agg_ps.tile([cfg.hid, 128], f32)
                slots = ([i * cfg.k_lo + j for j in range(cfg.k_lo)]
                         + [GKLO + i * cfg.k_hi + j for j in range(cfg.k_hi)])
                for jj, cs in enumerate(slots):
                    gc = g * NCHG + cs
                    cmat = c_p.tile([128, 128], bf16)
                    nc.vector.tensor_scalar(
                        cmat[:], iota_sb[:],
                        dstloc_sb[:, gc:gc + 1],
                        dinvdst_sb[:, gc:gc + 1],
                        mybir.AluOpType.is_equal,
                        mybir.AluOpType.mult)
                    nc.tensor.matmul(
                        agg[:], msgs[:, cs, 0:cfg.hid], cmat[:],
                        start=(jj == 0), stop=(jj == len(slots) - 1))
                relu = relu_p.tile([cfg.hid, 128], bf16)
                nc.scalar.activation(
                    relu[:], agg[:], mybir.ActivationFunctionType.Relu,
                    bias=b1_sb[:])
                o2 = o2_ps.tile([cfg.out_dim, 128], f32)
                nc.tensor.matmul(o2[:], wfcT_sb[:], relu[:],
                                 start=True, stop=True)
                if i == 0:
                    ostage = ost_p.tile([cfg.out_dim, G * 128], f32)
                nc.scalar.activation(
                    ostage[:, i * 128:(i + 1) * 128], o2[:],
                    mybir.ActivationFunctionType.Identity, bias=bfc_sb[:])
            nc.sync.dma_start(
                out_d[:, g * G * 128:(g + 1) * G * 128], ostage[:])

    nc.compile()
    return nc


# ----------------------------------------------------------------------------
# Entry points.
# ----------------------------------------------------------------------------

_CACHE = {}


def _get_compiled(edges, cfg_base):
    import hashlib
    e = np.ascontiguousarray(np.asarray(edges, dtype=np.int64))
    key = (e.shape, hashlib.sha1(e.tobytes()).hexdigest(), cfg_base)
    if key not in _CACHE:
        cfg, plan = _plan(e, cfg_base)
        nc = _build_module(cfg)
        _CACHE[key] = (cfg, plan, nc)
    return _CACHE[key]


def _run(X, edges, W1, b1, Wfc, bfc, cfg_base, trace=False):
    from concourse.bass_utils import run_bass_kernel_spmd

    cfg, plan, nc = _get_compiled(edges, cfg_base)
    in_maps = _make_inputs(X, W1, b1, Wfc, bfc, cfg, plan)
    res = run_bass_kernel_spmd(
        nc, in_maps, core_ids=list(range(cfg.n_cores)), trace=trace)

    s = cfg.slots_per_core
    full = np.concatenate([res.results[c]["out"] for c in range(cfg.n_cores)],
                          axis=1)                      # [40, n_slots]
    out = full[:, plan["node_to_slot"]].T.astype(np.float32)
    out = np.ascontiguousarray(out)
    return out, res


def kernel(X, edges, W1, b1, Wfc, bfc):
    cfg_base = (N_NODES, IN_DIM, HID, OUT_DIM, N_CORES, 49, 7, 32768)
    out, _ = _run(np.asarray(X, dtype=np.float32), np.asarray(edges),
                  np.asarray(W1, dtype=np.float32),
                  np.asarray(b1, dtype=np.float32),
                  np.asarray(Wfc, dtype=np.float32),
                  np.asarray(bfc, dtype=np.float32), cfg_base)
    return out



# revision 13
# speedup vs baseline: 1.1435x; 1.1435x over previous
"""Trainium2 Bass kernel for a 2-layer GCN (GCNConv -> ReLU -> Linear).

Math (matching the PyG-style reference):
    deg  = in_degree(dst) + 1 (self loops), dinv = deg^-1/2
    h    = X @ W1                                  [N, 64]
    agg[d] = dinv[d] * sum_{e:(s->d)} dinv[s]*h[s] (+ self loop)   [N, 64]
    out  = relu(agg + b1) @ Wfc.T + bfc            [N, 40]

Distribution over 8 NeuronCores (graph/data parallel), v2:
  - Nodes are packed into 392 tiles of <=128 slots balanced by real
    in-degree (so every tile has ~E/392 incoming edges); tiles are
    snake-assigned to (core, local_tile) by edge count so local tile i has
    nearly equal edge counts on every core (the compiled module is SPMD).
  - Phase 1 computes h' = dinv*h in two *pieces* (local tiles [0,25) and
    [25,49)); each piece is AllGathered as soon as it is ready so the
    collective overlaps the rest of phase 1 and the first gathers.  The
    table layout is piece-major then core-major, and each piece is
    < 32768 rows so gather indices fit int16 relative to the piece base.
  - Self-loop contributions never touch the gather path: per tile a
    diag(dinv) one-hot matmul seeded from the phase-1 SBUF stage opens the
    PSUM accumulation (saves ~6% of descriptors).
  - Remaining edges are fetched with bulk SWDGE dma_gather (256 B rows) in
    dst-tile-aligned 128-lane chunks (2 classes = the 2 src pieces); the
    per-(tile,class) chunk counts are the max over cores so the schedule
    is core-independent.  Trailing pad lanes get negative indices, which
    the gather ucode trims (no descriptor cost).
  - Scatter-add by destination is one-hot matmuls: C[j,d] =
    dinv_dst[j]*(dst_loc[j]==d) built on DVE with one fused tensor_scalar
    per chunk; relu(agg+b1) rides the PSUM eviction; layer 2 is one small
    matmul per tile; host un-permutes the [40, slots] outputs.
"""

import numpy as np

# ----------------------------------------------------------------------------
# Problem configuration (hardcoded; kernel.py must be self-contained).
# ----------------------------------------------------------------------------
N_NODES = 50000
N_EDGES = 800000
IN_DIM = 512
HID = 64
OUT_DIM = 40
N_CORES = 8

NT = 49                       # local tiles per core
N_TILES = N_CORES * NT        # 392
PIECE0 = 25                   # local tiles in piece 0 (piece 1 = NT-PIECE0)
GROUPS = (8, 8, 8, 8, 8, 7, 2)   # local tiles per phase-2 group
GMAX = 18                     # max chunks (128 idx each) per dma_gather call
TRIM_PADS = False             # negative trailing pad idxs (ucode trims them)

P0_ROWS = PIECE0 * 128                    # per-core rows in piece 0
P1_ROWS = (NT - PIECE0) * 128
BASE1 = N_CORES * P0_ROWS                 # table row where piece 1 starts
N_SLOTS = N_CORES * NT * 128              # 50176
SLOTS_PER_CORE = NT * 128                 # 6272
KD = IN_DIM // 128


class Plan:
    pass


def _plan(edges):
    """Host-side index/layout work (no feature math)."""
    src = np.asarray(edges[0], dtype=np.int64)
    dst = np.asarray(edges[1], dtype=np.int64)
    n = N_NODES

    rdeg = np.bincount(dst, minlength=n)           # real in-degree
    deg = rdeg + 1                                 # + self loop
    dinv = (1.0 / np.sqrt(deg.astype(np.float64))).astype(np.float32)

    # --- bin nodes into 392 tiles of <=128 slots, balancing in-edges ------
    order = np.argsort(-rdeg, kind="stable")
    ii = np.arange(n)
    rounds = ii // N_TILES
    pos = ii % N_TILES
    t_sorted = np.where(rounds % 2 == 0, pos, N_TILES - 1 - pos)
    assert rounds.max() < 128
    tile_of = np.empty(n, dtype=np.int64)
    lane_of = np.empty(n, dtype=np.int64)
    tile_of[order] = t_sorted
    lane_of[order] = rounds

    # --- tiles -> (core, local) snake by edge count -----------------------
    e_tile = np.bincount(tile_of[dst], minlength=N_TILES)
    trank = np.argsort(-e_tile, kind="stable")
    rr = np.arange(N_TILES)
    r16 = rr % 16
    core_r = np.where(r16 < 8, r16, 15 - r16)
    local_r = rr // 8
    core_of_tile = np.empty(N_TILES, dtype=np.int64)
    local_of_tile = np.empty(N_TILES, dtype=np.int64)
    core_of_tile[trank] = core_r
    local_of_tile[trank] = local_r
    assert local_of_tile.max() == NT - 1

    # --- table rows (piece-major, then core, then local tile, then lane) --
    c_n = core_of_tile[tile_of]
    lt_n = local_of_tile[tile_of]
    row_n = np.where(
        lt_n < PIECE0,
        c_n * P0_ROWS + lt_n * 128 + lane_of,
        BASE1 + c_n * P1_ROWS + (lt_n - PIECE0) * 128 + lane_of,
    )

    # per-core local slot (for X layout, stage, dinv, output)
    lslot_n = lt_n * 128 + lane_of

    slot_dinv = np.zeros((N_CORES, SLOTS_PER_CORE), dtype=np.float32)
    slot_dinv[c_n, lslot_n] = dinv

    # --- edge classification ---------------------------------------------
    ec = c_n[dst]                                  # owning (dst) core
    elt = lt_n[dst]                                # dst local tile
    edlane = lane_of[dst]                          # dst lane in tile
    ecls = (lt_n[src] >= PIECE0).astype(np.int64)  # src piece = class
    erel = row_n[src] - ecls * BASE1               # idx relative to class base
    assert erel.max() < 32768 and erel.min() >= 0
    edinv = dinv[dst]

    # counts per (core, local, class) -> uniform chunk counts k[local, cls]
    key = (ec * NT + elt) * 2 + ecls
    cnt = np.bincount(key, minlength=N_CORES * NT * 2).reshape(N_CORES, NT, 2)
    k_tc = np.maximum(1, (cnt.max(axis=0) + 127) // 128)      # [NT, 2]

    # --- chunk slot layout ------------------------------------------------
    # groups of local tiles; within a group: class-0 chunks (tile-major)
    # then class-1 chunks.  chunk_base[local, cls] = absolute chunk slot.
    group_of = np.empty(NT, dtype=np.int64)
    g_start = []
    t0 = 0
    for g, gn in enumerate(GROUPS):
        group_of[t0:t0 + gn] = g
        g_start.append(t0)
        t0 += gn
    assert t0 == NT

    chunk_base = np.zeros((NT, 2), dtype=np.int64)
    group_slot0 = []          # first chunk slot of each group
    group_calls = []          # per group: list of (cls, s0, s1) absolute
    slot = 0
    for g, gn in enumerate(GROUPS):
        group_slot0.append(slot)
        calls = []
        for cls in range(2):
            s0 = slot
            for i in range(g_start[g], g_start[g] + gn):
                chunk_base[i, cls] = slot
                slot += int(k_tc[i, cls])
            calls.append((cls, s0, slot))
        group_calls.append(calls)
    total_chunks = slot

    # --- place edges into (core-specific) chunk lanes ---------------------
    sort_idx = np.argsort(key, kind="stable")
    key_s = key[sort_idx]
    counts_flat = np.bincount(key_s, minlength=N_CORES * NT * 2)
    starts = np.concatenate([[0], np.cumsum(counts_flat)[:-1]])
    rank = np.arange(len(key_s)) - starts[key_s]

    e_slot = chunk_base[elt[sort_idx], ecls[sort_idx]] + (rank >> 7)
    e_lane = rank & 127

    idx16 = np.zeros((N_CORES, total_chunks, 128), dtype=np.int16)
    used = np.zeros((N_CORES, total_chunks, 128), dtype=bool)
    dstloc = np.full((N_CORES, total_chunks, 128), -1.0, dtype=np.float32)
    dinvdst = np.zeros((N_CORES, total_chunks, 128), dtype=np.float32)

    cidx = ec[sort_idx]
    idx16[cidx, e_slot, e_lane] = erel[sort_idx].astype(np.int16)
    used[cidx, e_slot, e_lane] = True
    dstloc[cidx, e_slot, e_lane] = edlane[sort_idx].astype(np.float32)
    dinvdst[cidx, e_slot, e_lane] = edinv[sort_idx]

    # --- trailing-pad trim: per gather call, mark trailing unused lanes ---
    # with negative idx so the ucode skips their descriptors entirely.
    call_plan = []            # flattened: (cls, cs, nch) chunk-slot ranges
    for g in range(len(GROUPS)):
        for (cls, s0, s1) in group_calls[g]:
            cs = s0
            while cs < s1:
                nch = min(GMAX, s1 - cs)
                call_plan.append((cls, cs, nch))
                cs += nch
    if TRIM_PADS:
        for (cls, cs, nch) in call_plan:
            u = used[:, cs:cs + nch, :].reshape(N_CORES, nch * 128)
            f = idx16[:, cs:cs + nch, :].reshape(N_CORES, nch * 128)
            for c in range(N_CORES):
                nz = np.nonzero(u[c])[0]
                last = nz[-1] + 1 if len(nz) else 0
                f[c, last:] = -1

    # --- pass table: diag cols then chunk cols ----------------------------
    n_pass = NT + total_chunks
    dl_all = np.empty((N_CORES, n_pass, 128), dtype=np.float32)
    di_all = np.empty((N_CORES, n_pass, 128), dtype=np.float32)
    lanes = np.arange(128, dtype=np.float32)
    for i in range(NT):
        dl_all[:, i, :] = lanes[None, :]
        di_all[:, i, :] = slot_dinv[:, i * 128:(i + 1) * 128]
    dl_all[:, NT:, :] = dstloc
    di_all[:, NT:, :] = dinvdst

    # --- wrap gather indices: flat position s -> partition s%16, col s//16,
    # replicated across the 8 q7 cores (128 partitions). ------------------
    n_idx_cols = total_chunks * 8
    flat = idx16.reshape(N_CORES, total_chunks * 128)
    wrapped = flat.reshape(N_CORES, n_idx_cols, 16).transpose(0, 2, 1)
    idx_wrapped = np.zeros((N_CORES, 128, n_idx_cols), dtype=np.int16)
    idx_wrapped[:, :16, :] = wrapped
    idx_wrapped[:, 16:, :] = np.tile(wrapped, (1, 7, 1))

    p = Plan()
    p.dinv = dinv
    p.slot_dinv = slot_dinv
    p.core_of_node = c_n
    p.lslot_of_node = lslot_n
    p.k_tc = k_tc
    p.total_chunks = total_chunks
    p.n_pass = n_pass
    p.chunk_base = chunk_base
    p.group_calls = group_calls
    p.group_slot0 = group_slot0
    p.g_start = g_start
    p.idx_wrapped = idx_wrapped
    p.dstloc = np.ascontiguousarray(dl_all.transpose(0, 2, 1))   # [C,128,np]
    p.dinvdst = np.ascontiguousarray(di_all.transpose(0, 2, 1))
    return p


def _make_inputs(X, W1, b1, Wfc, bfc, plan):
    import ml_dtypes
    bf16 = ml_dtypes.bfloat16

    Xp = np.zeros((N_CORES, SLOTS_PER_CORE, IN_DIM), dtype=np.float32)
    Xp[plan.core_of_node, plan.lslot_of_node] = np.asarray(X, dtype=np.float32)

    W1r = (np.asarray(W1, dtype=np.float32)
           .reshape(KD, 128, HID).transpose(1, 0, 2)
           .reshape(128, KD * HID).astype(bf16))
    wfcT = np.ascontiguousarray(np.asarray(Wfc, dtype=np.float32).T).astype(bf16)
    b1c = np.asarray(b1, dtype=np.float32).reshape(HID, 1)
    bfcc = np.asarray(bfc, dtype=np.float32).reshape(OUT_DIM, 1)
    iota = np.tile(np.arange(128, dtype=np.float32), (128, 1)).astype(bf16)

    in_maps = []
    for c in range(N_CORES):
        xt = np.ascontiguousarray(Xp[c].T).astype(bf16)
        dinv_sb = np.ascontiguousarray(
            plan.slot_dinv[c].reshape(NT, 128).T)
        in_maps.append({
            "xt": xt,
            "w1": W1r,
            "wfcT": wfcT,
            "b1": b1c,
            "bfc": bfcc,
            "iota": iota,
            "dinv_sb": dinv_sb,
            "idx": plan.idx_wrapped[c],
            "dstloc": plan.dstloc[c],
            "dinvdst": plan.dinvdst[c],
        })
    return in_maps


# ----------------------------------------------------------------------------
# Device kernel.
# ----------------------------------------------------------------------------

def _build_module(plan):
    import concourse.bass as bass
    import concourse.bacc as bacc
    import concourse.mybir as mybir
    import concourse.tile as tile
    from contextlib import ExitStack

    f32 = mybir.dt.float32
    bf16 = mybir.dt.bfloat16
    i16 = mybir.dt.int16
    S = SLOTS_PER_CORE
    n_idx_cols = plan.total_chunks * 8

    nc = bacc.Bacc("TRN2", target_bir_lowering=False, debug=False,
                   num_devices=N_CORES)

    xt_d = nc.dram_tensor("xt", [IN_DIM, S], bf16, kind="ExternalInput")
    w1_d = nc.dram_tensor("w1", [128, KD * HID], bf16, kind="ExternalInput")
    wfcT_d = nc.dram_tensor("wfcT", [HID, OUT_DIM], bf16,
                            kind="ExternalInput")
    b1_d = nc.dram_tensor("b1", [HID, 1], f32, kind="ExternalInput")
    bfc_d = nc.dram_tensor("bfc", [OUT_DIM, 1], f32, kind="ExternalInput")
    iota_d = nc.dram_tensor("iota", [128, 128], bf16, kind="ExternalInput")
    dinv_d = nc.dram_tensor("dinv_sb", [128, NT], f32, kind="ExternalInput")
    idx_d = nc.dram_tensor("idx", [128, n_idx_cols], i16, kind="ExternalInput")
    dstloc_d = nc.dram_tensor("dstloc", [128, plan.n_pass], f32,
                              kind="ExternalInput")
    dinvdst_d = nc.dram_tensor("dinvdst", [128, plan.n_pass], f32,
                               kind="ExternalInput")
    out_d = nc.dram_tensor("out", [OUT_DIM, S], f32, kind="ExternalOutput")

    with tile.TileContext(nc) as tc, ExitStack() as ctx:
        consts = ctx.enter_context(tc.tile_pool(name="consts", bufs=1))
        dram = ctx.enter_context(tc.tile_pool(name="dram", bufs=1,
                                              space="DRAM"))

        ag_in0 = dram.tile([P0_ROWS, 128], bf16, name="ag_in0")
        ag_in1 = dram.tile([P1_ROWS, 128], bf16, name="ag_in1")
        ag_out0 = dram.tile([N_CORES * P0_ROWS, 128], bf16, name="ag_out0")
        ag_out1 = dram.tile([N_CORES * P1_ROWS, 128], bf16, name="ag_out1")

        iota_sb = consts.tile([128, 128], bf16)
        w1_sb = consts.tile([128, KD * HID], bf16)
        wfcT_sb = consts.tile([HID, OUT_DIM], bf16)
        b1_sb = consts.tile([HID, 1], f32)
        bfc_sb = consts.tile([OUT_DIM, 1], f32)
        dinv_sb = consts.tile([128, NT], f32)
        idx_sb = consts.tile([128, n_idx_cols], i16)
        dstloc_sb = consts.tile([128, plan.n_pass], f32)
        dinvdst_sb = consts.tile([128, plan.n_pass], f32)
        stage = consts.tile([128, NT, 128], bf16)

        nc.sync.dma_start(iota_sb[:], iota_d[:])
        nc.sync.dma_start(w1_sb[:], w1_d[:])
        nc.sync.dma_start(wfcT_sb[:], wfcT_d[:])
        nc.sync.dma_start(b1_sb[:], b1_d[:])
        nc.sync.dma_start(bfc_sb[:], bfc_d[:])
        nc.sync.dma_start(dinv_sb[:], dinv_d[:])
        nc.sync.dma_start(idx_sb[:], idx_d[:])
        nc.sync.dma_start(dstloc_sb[:], dstloc_d[:])
        nc.sync.dma_start(dinvdst_sb[:], dinvdst_d[:])

        # ---- Phase 1 (per piece): h' = dinv * (X @ W1), AllGather --------
        pieces = ((0, PIECE0, ag_in0, ag_out0),
                  (PIECE0, NT, ag_in1, ag_out1))
        with tc.tile_pool(name="p1", bufs=2) as p1, \
                tc.tile_pool(name="p1ps", bufs=2, space="PSUM") as p1ps:
            for (t0, t1, ag_in, ag_out) in pieces:
                ncols = (t1 - t0) * 128
                xt_sb = p1.tile([128, KD, ncols], bf16)
                nc.sync.dma_start(
                    xt_sb[:],
                    xt_d[:, t0 * 128:t1 * 128]
                    .rearrange("(k p) s -> p k s", p=128))
                for t in range(t0, t1):
                    ph = p1ps.tile([128, HID], f32)
                    for k in range(KD):
                        nc.tensor.matmul(
                            ph[:],
                            xt_sb[:, k, (t - t0) * 128:(t - t0 + 1) * 128],
                            w1_sb[:, k * HID:(k + 1) * HID],
                            start=(k == 0), stop=(k == KD - 1))
                    nc.vector.tensor_scalar_mul(
                        stage[:, t, 0:HID], ph[:], dinv_sb[:, t:t + 1])
                nc.sync.dma_start(
                    ag_in[:].rearrange("(t p) e -> p t e", p=128),
                    stage[:, t0:t1, :])
                nc.gpsimd.collective_compute(
                    "AllGather",
                    mybir.AluOpType.bypass,
                    ins=[ag_in.opt()],
                    outs=[ag_out.opt()],
                    replica_groups=[list(range(N_CORES))],
                )

        # ---- Phase 2: gather + one-hot scatter matmuls + layer 2 ---------
        msgs_p = ctx.enter_context(tc.tile_pool(name="msgs", bufs=2))
        c_p = ctx.enter_context(tc.tile_pool(name="cmat", bufs=8))
        relu_p = ctx.enter_context(tc.tile_pool(name="relu", bufs=3))
        ost_p = ctx.enter_context(tc.tile_pool(name="ost", bufs=2))
        agg_ps = ctx.enter_context(
            tc.tile_pool(name="aggps", bufs=4, space="PSUM"))
        o2_ps = ctx.enter_context(
            tc.tile_pool(name="o2ps", bufs=2, space="PSUM"))

        nchg_max = max(
            sum(s1 - s0 for (_, s0, s1) in plan.group_calls[g])
            for g in range(len(GROUPS)))

        tbls = (ag_out0[:], ag_out1[:])

        for g, gn in enumerate(GROUPS):
            gslot0 = plan.group_slot0[g]
            msgs = msgs_p.tile([128, nchg_max, 128], bf16)
            if g < 2:
                nc.vector.memset(msgs[:], 0.0)
            for (cls, s0, s1) in plan.group_calls[g]:
                cs = s0
                while cs < s1:
                    nch = min(GMAX, s1 - cs)
                    nc.gpsimd.dma_gather(
                        msgs[:, cs - gslot0:cs - gslot0 + nch, :],
                        tbls[cls],
                        idx_sb[:, cs * 8:(cs + nch) * 8],
                        nch * 128, nch * 128, 128,
                        single_packet=False)
                    cs += nch

            t0 = plan.g_start[g]
            for t in range(t0, t0 + gn):
                agg = agg_ps.tile([HID, 128], f32)
                passes = [(None, t)]
                for cls in range(2):
                    cb = int(plan.chunk_base[t, cls])
                    for j in range(int(plan.k_tc[t, cls])):
                        passes.append((cb + j, NT + cb + j))
                for j, (slot, col) in enumerate(passes):
                    cmat = c_p.tile([128, 128], bf16)
                    nc.vector.tensor_scalar(
                        cmat[:], iota_sb[:],
                        dstloc_sb[:, col:col + 1],
                        dinvdst_sb[:, col:col + 1],
                        mybir.AluOpType.is_equal,
                        mybir.AluOpType.mult)
                    if slot is None:
                        lhs = stage[:, t, 0:HID]
                    else:
                        lhs = msgs[:, slot - gslot0, 0:HID]
                    nc.tensor.matmul(
                        agg[:], lhs, cmat[:],
                        start=(j == 0), stop=(j == len(passes) - 1))
                relu = relu_p.tile([HID, 128], bf16)
                nc.scalar.activation(
                    relu[:], agg[:], mybir.ActivationFunctionType.Relu,
                    bias=b1_sb[:])
                o2 = o2_ps.tile([OUT_DIM, 128], f32)
                nc.tensor.matmul(o2[:], wfcT_sb[:], relu[:],
                                 start=True, stop=True)
                if t == t0:
                    ostage = ost_p.tile([OUT_DIM, gn * 128], f32)
                nc.scalar.activation(
                    ostage[:, (t - t0) * 128:(t - t0 + 1) * 128], o2[:],
                    mybir.ActivationFunctionType.Identity, bias=bfc_sb[:])
            nc.sync.dma_start(
                out_d[:, t0 * 128:(t0 + gn) * 128], ostage[:])

    nc.compile()
    return nc


# ----------------------------------------------------------------------------
# Entry points.
# ----------------------------------------------------------------------------

_CACHE = {}


def _get_compiled(edges):
    import hashlib
    e = np.ascontiguousarray(np.asarray(edges, dtype=np.int64))
    key = (e.shape, hashlib.sha1(e.tobytes()).hexdigest())
    if key not in _CACHE:
        plan = _plan(e)
        nc = _build_module(plan)
        _CACHE[key] = (plan, nc)
    return _CACHE[key]


def _run(X, edges, W1, b1, Wfc, bfc, trace=False):
    from concourse.bass_utils import run_bass_kernel_spmd

    plan, nc = _get_compiled(edges)
    in_maps = _make_inputs(X, W1, b1, Wfc, bfc, plan)
    res = run_bass_kernel_spmd(
        nc, in_maps, core_ids=list(range(N_CORES)), trace=trace)

    full = np.stack([res.results[c]["out"] for c in range(N_CORES)])
    # full[c, :, lslot] -> node
    out = full[plan.core_of_node, :, plan.lslot_of_node].astype(np.float32)
    out = np.ascontiguousarray(out)
    return out, res


def kernel(X, edges, W1, b1, Wfc, bfc):
    out, _ = _run(np.asarray(X, dtype=np.float32), np.asarray(edges),
                  np.asarray(W1, dtype=np.float32),
                  np.asarray(b1, dtype=np.float32),
                  np.asarray(Wfc, dtype=np.float32),
                  np.asarray(bfc, dtype=np.float32))
    return out
